# revision 24
# baseline (speedup 1.0000x reference)
"""GraphSAGE 2-layer forward on 8 Trainium2 NeuronCores.

Strategy (sharding_hint: partition edges by destination node):
  - Nodes are padded to NP=50176 = 8 cores * 49 chunks * 128 and sharded by
    destination across the 8 cores (6272 dst nodes per core).
  - Edges are bucketed by dst chunk (128 dst nodes per chunk).  Within a
    bucket, edges are split into src<32768 ("lo") and src>=32768 ("hi")
    sub-lists because dma_gather indices are int16.
  - Per chunk, messages x[src] are fetched with dma_gather (one instruction
    per lo/hi block) in bf16 (256B descriptors), and aggregated with one-hot
    matmuls on the tensor engine.  One-hot (0/1) selection matrices for a
    whole chunk (up to TTmax tiles) are built in a single batched DVE
    tensor_tensor is_equal op against a broadcast drel table; the 1/deg mean
    scaling is applied after aggregation (free-dim inv_rep multiply for
    layer 1, per-partition scalar multiply for layer 2).
  - Layer 2 gathers h @ W2_l (= p, 64 wide) stored padded to 128 bf16 lanes
    so the same int16 index tables are reused; p is exchanged between cores
    with an AllGather.
  - All compute matmuls run in bf16 (messages, one-hots, weights) with fp32
    PSUM accumulation.
"""

import sys

sys.path.insert(0, "/opt/trn_rl_repo")

import numpy as np

N = 50000
E = 800000
D_IN, D_HID, D_OUT = 128, 128, 64
N_CORES = 8
CHUNK = 128
C_PER_CORE = 49
NODES_PC = C_PER_CORE * CHUNK  # 6272
NP_ = N_CORES * NODES_PC  # 50176
NCH = N_CORES * C_PER_CORE  # 392 chunks
SPLIT = 32768
GROUP = 1  # chunks per merged dma_gather pair (1 = per-chunk gathers)
# layer-2 p exchange is split into two AllGathers so the first overlaps
# phase A: block A = local chunks [0, CA), block B = [CA, 49)
CA = 25
ROWS_A = CA * CHUNK  # 3200 rows per core
ROWS_B = NODES_PC - ROWS_A  # 3072


def _preprocess(x, edge_index):
    """Host-side index preprocessing. Returns per-core input maps + profile."""
    import ml_dtypes

    x = np.asarray(x, dtype=np.float32)
    src = np.asarray(edge_index[0], dtype=np.int64)
    dst = np.asarray(edge_index[1], dtype=np.int64)

    cnt = np.bincount(dst, minlength=NP_).astype(np.float32)
    inv = (1.0 / np.maximum(cnt, 1.0)).astype(np.float32)

    chunk = dst // CHUNK
    hi_flag = (src >= SPLIT).astype(np.int64)
    order = np.lexsort((hi_flag, chunk))
    s_src = src[order]
    s_dst = dst[order]

    lo_mask = src < SPLIT
    n_lo = np.bincount(chunk[lo_mask], minlength=NCH)
    n_hi = np.bincount(chunk[~lo_mask], minlength=NCH)
    n_tot = n_lo + n_hi
    start = np.zeros(NCH, np.int64)
    start[1:] = np.cumsum(n_tot)[:-1]

    t_lo_c = -(-n_lo // 128)  # ceil
    t_hi_c = -(-n_hi // 128)
    # slot profile: max tile count over the 8 cores for each of 49 slots
    TL = t_lo_c.reshape(N_CORES, C_PER_CORE).max(axis=0)
    TH = t_hi_c.reshape(N_CORES, C_PER_CORE).max(axis=0)
    TT = TL + TH
    T_total = int(TT.sum())
    S = T_total * 128  # total gather slots per core per layer

    x_pad = np.zeros((NP_, D_IN), np.float32)
    x_pad[:N] = x
    x_g16 = x_pad.astype(ml_dtypes.bfloat16)

    # inv_rep: [128, NODES_PC] per core, row-replicated inverse degree (bf16)
    # inv_colT: [128, C_PER_CORE] per core, per-dst-lane inverse degree (f32)
    # pair layout: for chunk pair (2q, 2q+1) the slot order is
    # [lo(2q) | lo(2q+1) | hi(2q) | hi(2q+1)] so one dma_gather covers both
    # chunks' lo (resp. hi) tiles.  Odd trailing chunk keeps [lo | hi].
    per_core = []
    for k in range(N_CORES):
        idx16 = np.zeros(S, np.int16)
        drel = np.full(S, 200.0, np.float32)
        pos = 0
        for q in range(0, C_PER_CORE, GROUP):
            js = list(range(q, min(q + GROUP, C_PER_CORE)))
            for j in js:  # lo blocks
                c = k * C_PER_CORE + j
                nl = int(n_lo[c])
                s0 = int(start[c])
                idx16[pos : pos + nl] = s_src[s0 : s0 + nl]
                drel[pos : pos + nl] = s_dst[s0 : s0 + nl] % CHUNK
                pos += int(TL[j]) * 128
            for j in js:  # hi blocks
                c = k * C_PER_CORE + j
                nl, nh = int(n_lo[c]), int(n_hi[c])
                s0 = int(start[c])
                idx16[pos : pos + nh] = s_src[s0 + nl : s0 + nl + nh] - SPLIT
                drel[pos : pos + nh] = s_dst[s0 + nl : s0 + nl + nh] % CHUNK
                pos += int(TH[j]) * 128
        assert pos == S
        idx_wrapped = np.ascontiguousarray(
            np.tile(idx16.reshape(S // 16, 16).T, (8, 1))
        )  # [128, S/16]
        drel2 = np.ascontiguousarray(
            drel.reshape(T_total, 128).T.astype(ml_dtypes.bfloat16)
        )  # [128, T]
        inv_k = inv[k * NODES_PC : (k + 1) * NODES_PC]
        inv_rep = np.ascontiguousarray(
            np.tile(inv_k[None, :], (128, 1)).astype(ml_dtypes.bfloat16)
        )  # [128, NODES_PC]
        inv_colT = np.ascontiguousarray(
            inv_k.reshape(C_PER_CORE, 128).T.astype(np.float32)
        )  # [128, 49]
        xT_k = np.ascontiguousarray(
            x_pad[k * NODES_PC : (k + 1) * NODES_PC].T.astype(ml_dtypes.bfloat16)
        )
        per_core.append(
            {
                "x_g": x_g16,
                "xT": xT_k,
                "idx": idx_wrapped,
                "drel": drel2,
                "inv_rep": inv_rep,
                "inv_colT": inv_colT,
            }
        )

    # ---- layer-2 tables: edges split by A/B block membership of the src ----
    src_r = s_src // NODES_PC
    src_off = s_src % NODES_PC
    isB = src_off >= ROWS_A
    rowAB = np.where(isB, src_r * ROWS_B + (src_off - ROWS_A),
                     src_r * ROWS_A + src_off)
    chunk_s = s_dst // CHUNK  # chunk of each edge in (chunk, lo/hi) order
    order2 = np.lexsort((isB, chunk_s))
    t_src = rowAB[order2]
    t_dst = s_dst[order2]
    t_isB = isB[order2]
    n_A = np.bincount(chunk_s[~isB], minlength=NCH)
    n_B = np.bincount(chunk_s[isB], minlength=NCH)
    TA = (-(-n_A // 128)).reshape(N_CORES, C_PER_CORE).max(axis=0)
    TB = (-(-n_B // 128)).reshape(N_CORES, C_PER_CORE).max(axis=0)
    T2 = int(TA.sum() + TB.sum())
    S2 = T2 * 128
    for k in range(N_CORES):
        idx16 = np.zeros(S2, np.int16)
        drel = np.full(S2, 200.0, np.float32)
        pos = 0
        for j in range(C_PER_CORE):
            c = k * C_PER_CORE + j
            na, nb = int(n_A[c]), int(n_B[c])
            s0 = int(start[c])
            idx16[pos : pos + na] = t_src[s0 : s0 + na]
            drel[pos : pos + na] = t_dst[s0 : s0 + na] % CHUNK
            assert not t_isB[s0 : s0 + na].any()
            pos += int(TA[j]) * 128
            idx16[pos : pos + nb] = t_src[s0 + na : s0 + na + nb]
            drel[pos : pos + nb] = t_dst[s0 + na : s0 + na + nb] % CHUNK
            assert t_isB[s0 + na : s0 + na + nb].all()
            pos += int(TB[j]) * 128
        assert pos == S2
        per_core[k]["idx2"] = np.ascontiguousarray(
            np.tile(idx16.reshape(S2 // 16, 16).T, (8, 1))
        )
        per_core[k]["drel2"] = np.ascontiguousarray(
            drel.reshape(T2, 128).T.astype(ml_dtypes.bfloat16)
        )
    return (
        per_core,
        [int(v) for v in TL],
        [int(v) for v in TH],
        [int(v) for v in TA],
        [int(v) for v in TB],
    )


def _shared_inputs(W1_l, b1, W1_r, W2_l, b2, W2_r):
    import ml_dtypes

    bf = ml_dtypes.bfloat16
    return {
        "W1_l": np.ascontiguousarray(np.asarray(W1_l, np.float32).astype(bf)),
        "W1_r": np.ascontiguousarray(np.asarray(W1_r, np.float32).astype(bf)),
        "W2_l": np.ascontiguousarray(np.asarray(W2_l, np.float32).astype(bf)),
        "W2_r": np.ascontiguousarray(np.asarray(W2_r, np.float32).astype(bf)),
        "b1": np.ascontiguousarray(np.asarray(b1, np.float32).reshape(D_HID, 1)),
        "b2": np.ascontiguousarray(
            np.asarray(b2, np.float32).astype(bf).reshape(1, D_OUT)
        ),
    }


def _build(TL, TH, TA, TB, n_chunks=C_PER_CORE):
    import concourse.bacc as bacc
    import concourse.mybir as mybir
    from concourse.tile import TileContext

    f32 = mybir.dt.float32
    bf16 = mybir.dt.bfloat16
    i16 = mybir.dt.int16
    TT = [a + b for a, b in zip(TL, TH)]
    T_total = sum(TT)
    S16 = T_total * 8  # idx table columns (16 idx per column)

    # chunk groups; col layout per group [lo..los | hi..his]
    pairs = [
        list(range(q, min(q + GROUP, n_chunks)))
        for q in range(0, n_chunks, GROUP)
    ]
    TT2 = [a + b for a, b in zip(TA, TB)]
    T2_total = sum(TT2)
    S16b = T2_total * 8
    PTmax = max(
        max(sum(TT[j] for j in js) for js in pairs),
        max(sum(TT2[j] for j in js) for js in pairs),
    )

    nc = bacc.Bacc(
        "TRN2",
        target_bir_lowering=False,
        debug=False,
        enable_asserts=False,
        num_devices=N_CORES,
    )

    x_g = nc.dram_tensor("x_g", [NP_, D_IN], bf16, kind="ExternalInput").ap()
    xT_d = nc.dram_tensor("xT", [128, NODES_PC], bf16, kind="ExternalInput").ap()
    idx_d = nc.dram_tensor("idx", [128, S16], i16, kind="ExternalInput").ap()
    drel_d = nc.dram_tensor("drel", [128, T_total], bf16, kind="ExternalInput").ap()
    invr_d = nc.dram_tensor(
        "inv_rep", [128, NODES_PC], bf16, kind="ExternalInput"
    ).ap()
    invc_d = nc.dram_tensor(
        "inv_colT", [128, C_PER_CORE], f32, kind="ExternalInput"
    ).ap()
    w1l_d = nc.dram_tensor("W1_l", [D_IN, D_HID], bf16, kind="ExternalInput").ap()
    w1r_d = nc.dram_tensor("W1_r", [D_IN, D_HID], bf16, kind="ExternalInput").ap()
    w2l_d = nc.dram_tensor("W2_l", [D_HID, D_OUT], bf16, kind="ExternalInput").ap()
    w2r_d = nc.dram_tensor("W2_r", [D_HID, D_OUT], bf16, kind="ExternalInput").ap()
    b1_d = nc.dram_tensor("b1", [D_HID, 1], f32, kind="ExternalInput").ap()
    b2_d = nc.dram_tensor("b2", [1, D_OUT], bf16, kind="ExternalInput").ap()
    idx2_d = nc.dram_tensor("idx2", [128, S16b], i16, kind="ExternalInput").ap()
    drel2_d = nc.dram_tensor(
        "drel2", [128, T2_total], bf16, kind="ExternalInput"
    ).ap()
    out_d = nc.dram_tensor("out", [NODES_PC, D_OUT], f32, kind="ExternalOutput").ap()
    p_full = nc.dram_tensor(
        "p_full", [NP_, 128], bf16, kind="Internal", addr_space="Shared"
    ).ap()

    relu = mybir.ActivationFunctionType.Relu
    is_eq = mybir.AluOpType.is_equal
    mult = mybir.AluOpType.mult
    add = mybir.AluOpType.add

    with TileContext(nc) as tc:
        with (
            tc.tile_pool(name="persist", bufs=1) as pp,
            tc.tile_pool(name="dram", bufs=1, space="DRAM") as dp,
            tc.tile_pool(name="msg", bufs=2) as mpool,
            tc.tile_pool(name="oh", bufs=3) as ohpool,
            tc.tile_pool(name="stage", bufs=3) as spool,
            tc.tile_pool(name="psA", bufs=2, space="PSUM") as psA,
            tc.tile_pool(name="psH", bufs=2, space="PSUM") as psH,
            tc.tile_pool(name="psO", bufs=2, space="PSUM") as psO,
        ):
            xT_sb = pp.tile([128, NODES_PC], bf16)
            nc.sync.dma_start(out=xT_sb[:], in_=xT_d)
            idx_sb = pp.tile([128, S16], i16)
            nc.sync.dma_start(out=idx_sb[:], in_=idx_d)
            drel_sb = pp.tile([128, T_total], bf16)
            nc.sync.dma_start(out=drel_sb[:], in_=drel_d)
            idx2_sb = pp.tile([128, S16b], i16)
            nc.sync.dma_start(out=idx2_sb[:], in_=idx2_d)
            drel2_sb = pp.tile([128, T2_total], bf16)
            nc.sync.dma_start(out=drel2_sb[:], in_=drel2_d)
            invr_sb = pp.tile([128, NODES_PC], bf16)
            nc.sync.dma_start(out=invr_sb[:], in_=invr_d)
            invc_sb = pp.tile([128, C_PER_CORE], f32)
            nc.sync.dma_start(out=invc_sb[:], in_=invc_d)
            w1l_sb = pp.tile([D_IN, D_HID], bf16)
            nc.sync.dma_start(out=w1l_sb[:], in_=w1l_d)
            w1r_sb = pp.tile([D_IN, D_HID], bf16)
            nc.sync.dma_start(out=w1r_sb[:], in_=w1r_d)
            w2l_sb = pp.tile([D_HID, D_OUT], bf16)
            nc.sync.dma_start(out=w2l_sb[:], in_=w2l_d)
            w2r_sb = pp.tile([D_HID, D_OUT], bf16)
            nc.sync.dma_start(out=w2r_sb[:], in_=w2r_d)
            b1_sb = pp.tile([D_HID, 1], f32)
            nc.sync.dma_start(out=b1_sb[:], in_=b1_d)
            b2_sb = pp.tile([1, D_OUT], bf16)
            nc.sync.dma_start(out=b2_sb[:], in_=b2_d)
            iota_sb = pp.tile([128, 128], f32)
            nc.gpsimd.iota(
                iota_sb[:],
                pattern=[[1, 128]],
                base=0,
                channel_multiplier=0,
                allow_small_or_imprecise_dtypes=True,
            )
            iota16 = pp.tile([128, 128], bf16)
            nc.vector.tensor_copy(out=iota16[:], in_=iota_sb[:])
            iota_rep = pp.tile([128, PTmax * 128], bf16)
            for t in range(PTmax):
                nc.scalar.copy(
                    out=iota_rep[:, t * 128 : (t + 1) * 128], in_=iota16[:]
                )
            ones_sb = pp.tile([1, 128], bf16)
            nc.vector.memset(ones_sb[:], 1.0)
            h_all = pp.tile([128, NODES_PC], bf16)
            p_bounce = dp.tile([NODES_PC, 128], bf16)

            # ---------------- phase A: layer 1 + p = h @ W2_l ----------------
            tb = 0
            for js in pairs:
                L = [TL[j] for j in js]
                H = [TH[j] for j in js]
                sl, sh = sum(L), sum(H)
                ttp = sl + sh
                msg = mpool.tile([128, PTmax * 128], bf16, tag="msg")
                if sl:
                    nc.gpsimd.dma_gather(
                        out_ap=msg[:, : sl * 128].rearrange("p (t e) -> p t e", e=128),
                        in_ap=x_g[0:SPLIT, :],
                        idxs_ap=idx_sb[:, tb * 8 : (tb + sl) * 8],
                        num_idxs=sl * 128,
                        num_idxs_reg=sl * 128,
                        elem_size=128,
                        single_packet=False,
                    )
                if sh:
                    nc.gpsimd.dma_gather(
                        out_ap=msg[:, sl * 128 : ttp * 128].rearrange(
                            "p (t e) -> p t e", e=128
                        ),
                        in_ap=x_g[SPLIT:NP_, :],
                        idxs_ap=idx_sb[:, (tb + sl) * 8 : (tb + ttp) * 8],
                        num_idxs=sh * 128,
                        num_idxs_reg=sh * 128,
                        elem_size=128,
                        single_packet=False,
                    )
                # batched one-hot build: oh[e, (t,d)] = (iota[d] == drel[e,t])
                oh = ohpool.tile([128, PTmax * 128], bf16, tag="oh")
                nc.vector.tensor_tensor(
                    out=oh[:, : ttp * 128].rearrange("p (t e) -> p t e", e=128),
                    in0=iota_rep[:, : ttp * 128].rearrange("p (t e) -> p t e", e=128),
                    in1=drel_sb[:, tb : tb + ttp]
                    .rearrange("p (t e) -> p t e", e=1)
                    .broadcast_to([128, ttp, 128]),
                    op=is_eq,
                )
                for i, j in enumerate(js):
                    # chunk j's tile columns within the pair buffer
                    cols = list(range(sum(L[:i]), sum(L[: i + 1]))) + list(
                        range(sl + sum(H[:i]), sl + sum(H[: i + 1]))
                    )
                    pa = psA.tile([128, 128], f32, tag="agg")
                    for ci, t in enumerate(cols):
                        nc.tensor.matmul(
                            out=pa[:],
                            lhsT=msg[:, t * 128 : (t + 1) * 128],
                            rhs=oh[:, t * 128 : (t + 1) * 128],
                            start=(ci == 0),
                            stop=(ci == len(cols) - 1),
                        )
                    # meanT[f, d] = aggT * inv_deg[d] (free-dim scale, inv_rep)
                    jsl = slice(j * 128, (j + 1) * 128)
                    meanT = spool.tile([128, 128], bf16, tag="meanT")
                    nc.vector.tensor_tensor(
                        out=meanT[:], in0=pa[:], in1=invr_sb[:, jsl], op=mult
                    )
                    ph = psH.tile([128, 128], f32, tag="h")
                    nc.tensor.matmul(
                        out=ph[:], lhsT=w1l_sb[:], rhs=meanT[:], start=True, stop=False
                    )
                    nc.tensor.matmul(
                        out=ph[:], lhsT=w1r_sb[:], rhs=xT_sb[:, jsl],
                        start=False, stop=True,
                    )
                    nc.scalar.activation(
                        out=h_all[:, jsl], in_=ph[:], func=relu,
                        bias=b1_sb[:, 0:1], scale=1.0,
                    )
                    po = psO.tile([128, D_OUT], f32, tag="p")
                    nc.tensor.matmul(
                        out=po[:], lhsT=h_all[:, jsl], rhs=w2l_sb[:],
                        start=True, stop=True,
                    )
                    p_sb = spool.tile([128, 128], bf16, tag="p_sb")
                    nc.vector.memset(p_sb[:, D_OUT:128], 0.0)
                    nc.scalar.copy(out=p_sb[:, 0:D_OUT], in_=po[:])
                    nc.sync.dma_start(out=p_bounce[jsl, :], in_=p_sb[:])
                tb += ttp

            # ---------------- all-gather p ----------------
            nc.gpsimd.collective_compute(
                "AllGather",
                mybir.AluOpType.bypass,
                replica_groups=[list(range(N_CORES))],
                ins=[p_bounce[:]],
                outs=[p_full],
            )

            # ---------------- phase B: layer 2 ----------------
            tb = 0
            for js in pairs:
                L = [TL[j] for j in js]
                H = [TH[j] for j in js]
                sl, sh = sum(L), sum(H)
                ttp = sl + sh
                msg2 = mpool.tile([128, PTmax * 128], bf16, tag="msg2")
                if sl:
                    nc.gpsimd.dma_gather(
                        out_ap=msg2[:, : sl * 128].rearrange(
                            "p (t e) -> p t e", e=128
                        ),
                        in_ap=p_full[0:SPLIT, :],
                        idxs_ap=idx_sb[:, tb * 8 : (tb + sl) * 8],
                        num_idxs=sl * 128,
                        num_idxs_reg=sl * 128,
                        elem_size=128,
                        single_packet=False,
                    )
                if sh:
                    nc.gpsimd.dma_gather(
                        out_ap=msg2[:, sl * 128 : ttp * 128].rearrange(
                            "p (t e) -> p t e", e=128
                        ),
                        in_ap=p_full[SPLIT:NP_, :],
                        idxs_ap=idx_sb[:, (tb + sl) * 8 : (tb + ttp) * 8],
                        num_idxs=sh * 128,
                        num_idxs_reg=sh * 128,
                        elem_size=128,
                        single_packet=False,
                    )
                oh = ohpool.tile([128, PTmax * 128], bf16, tag="oh")
                nc.vector.tensor_tensor(
                    out=oh[:, : ttp * 128].rearrange("p (t e) -> p t e", e=128),
                    in0=iota_rep[:, : ttp * 128].rearrange("p (t e) -> p t e", e=128),
                    in1=drel_sb[:, tb : tb + ttp]
                    .rearrange("p (t e) -> p t e", e=1)
                    .broadcast_to([128, ttp, 128]),
                    op=is_eq,
                )
                for i, j in enumerate(js):
                    cols = list(range(sum(L[:i]), sum(L[: i + 1]))) + list(
                        range(sl + sum(H[:i]), sl + sum(H[: i + 1]))
                    )
                    jsl = slice(j * 128, (j + 1) * 128)
                    # agg2[d, p-feat] = sum_e oh[e, d] * msg2[e, p]
                    pf = psA.tile([128, 128], f32, tag="fin")
                    for ci, t in enumerate(cols):
                        nc.tensor.matmul(
                            out=pf[:],
                            lhsT=oh[:, t * 128 : (t + 1) * 128],
                            rhs=msg2[:, t * 128 : (t + 1) * 128],
                            start=(ci == 0),
                            stop=(ci == len(cols) - 1),
                        )
                    # dense part: h @ W2_r + b2  -> pd [128d, 64]
                    pd = psO.tile([128, D_OUT], f32, tag="p")
                    nc.tensor.matmul(
                        out=pd[:], lhsT=h_all[:, jsl], rhs=w2r_sb[:],
                        start=True, stop=False,
                    )
                    nc.tensor.matmul(
                        out=pd[:], lhsT=ones_sb[:], rhs=b2_sb[:],
                        start=False, stop=True,
                    )
                    # out = pf[:, :64] * inv_col + pd
                    pd_sb = spool.tile([128, D_OUT], f32, tag="pd_sb")
                    nc.scalar.copy(out=pd_sb[:], in_=pd[:])
                    out_sb = spool.tile([128, D_OUT], f32, tag="out_sb")
                    nc.vector.scalar_tensor_tensor(
                        out=out_sb[:],
                        in0=pf[:, 0:D_OUT],
                        scalar=invc_sb[:, j : j + 1],
                        in1=pd_sb[:],
                        op0=mult,
                        op1=add,
                    )
                    nc.sync.dma_start(out=out_d[jsl, :], in_=out_sb[:])
                tb += ttp

    nc.compile()
    return nc


def kernel(
    x,
    edge_index,
    W1_l,
    b1,
    W1_r,
    W2_l,
    b2,
    W2_r,
):
    from concourse.bass_utils import run_bass_kernel_spmd

    per_core, TL, TH, TA, TB = _preprocess(x, edge_index)
    nc = _build(TL, TH, TA, TB)

    shared = _shared_inputs(W1_l, b1, W1_r, W2_l, b2, W2_r)
    in_maps = [{**pc, **shared} for pc in per_core]

    res = run_bass_kernel_spmd(nc, in_maps, core_ids=list(range(N_CORES)))
    out = np.concatenate([r["out"] for r in res.results], axis=0)
    return out[:N].astype(np.float32)


if __name__ == "__main__":
    rng = np.random.default_rng(0)
    x = rng.standard_normal((N, D_IN), dtype=np.float32)
    ei = rng.integers(0, N, size=(2, E), dtype=np.int64)
    s = 1.0 / np.sqrt(D_IN)
    w1l = rng.uniform(-s, s, (D_IN, D_HID)).astype(np.float32)
    w1r = rng.uniform(-s, s, (D_IN, D_HID)).astype(np.float32)
    s2 = 1.0 / np.sqrt(D_HID)
    w2l = rng.uniform(-s2, s2, (D_HID, D_OUT)).astype(np.float32)
    w2r = rng.uniform(-s2, s2, (D_HID, D_OUT)).astype(np.float32)
    out = kernel(
        x=x,
        edge_index=ei,
        W1_l=w1l,
        b1=np.zeros(D_HID, np.float32),
        W1_r=w1r,
        W2_l=w2l,
        b2=np.zeros(D_OUT, np.float32),
        W2_r=w2r,
    )
    print(out.shape, out.dtype)


# revision 27
# speedup vs baseline: 1.1685x; 1.1685x over previous
"""GraphSAGE 2-layer forward on 8 Trainium2 NeuronCores.

Strategy (sharding_hint: partition edges by destination node):
  - Nodes are padded to NP=50176 = 8 cores * 49 chunks * 128 and sharded by
    destination across the 8 cores (6272 dst nodes per core).
  - Edges are bucketed by dst chunk (128 dst nodes per chunk).  Within a
    bucket, edges are split into src<32768 ("lo") and src>=32768 ("hi")
    sub-lists because dma_gather indices are int16.
  - Per chunk, messages x[src] are fetched with dma_gather (one instruction
    per lo/hi block) in bf16 (256B descriptors), and aggregated with one-hot
    matmuls on the tensor engine.  One-hot (0/1) selection matrices for a
    whole chunk (up to TTmax tiles) are built in a single batched DVE
    tensor_tensor is_equal op against a broadcast drel table; the 1/deg mean
    scaling is applied after aggregation (free-dim inv_rep multiply for
    layer 1, per-partition scalar multiply for layer 2).
  - Layer 2 gathers h @ W2_l (= p, 64 wide) stored padded to 128 bf16 lanes
    so the same int16 index tables are reused; p is exchanged between cores
    with an AllGather.
  - All compute matmuls run in bf16 (messages, one-hots, weights) with fp32
    PSUM accumulation.
"""

import sys

sys.path.insert(0, "/opt/trn_rl_repo")

import numpy as np

N = 50000
E = 800000
D_IN, D_HID, D_OUT = 128, 128, 64
N_CORES = 8
CHUNK = 128
C_PER_CORE = 49
NODES_PC = C_PER_CORE * CHUNK  # 6272
NP_ = N_CORES * NODES_PC  # 50176
NCH = N_CORES * C_PER_CORE  # 392 chunks
SPLIT = 32768
GROUP = 2  # chunks per slot-layout group: lo gathers stay per-chunk (~1340
# idx, merging would exceed the ~2048-idx SWDGE ring sweet spot) while the
# smaller hi gathers (~710 idx) are merged per group (~1420 idx)
# layer-2 p exchange is split into two AllGathers so the first overlaps
# phase A: block A = local chunks [0, CA), block B = [CA, 49)
CA = 25
ROWS_A = CA * CHUNK  # 3200 rows per core
ROWS_B = NODES_PC - ROWS_A  # 3072


def _preprocess(x, edge_index):
    """Host-side index preprocessing. Returns per-core input maps + profile."""
    import ml_dtypes

    x = np.asarray(x, dtype=np.float32)
    src = np.asarray(edge_index[0], dtype=np.int64)
    dst = np.asarray(edge_index[1], dtype=np.int64)

    cnt = np.bincount(dst, minlength=NP_).astype(np.float32)
    inv = (1.0 / np.maximum(cnt, 1.0)).astype(np.float32)

    chunk = dst // CHUNK
    hi_flag = (src >= SPLIT).astype(np.int64)
    order = np.lexsort((hi_flag, chunk))
    s_src = src[order]
    s_dst = dst[order]

    lo_mask = src < SPLIT
    n_lo = np.bincount(chunk[lo_mask], minlength=NCH)
    n_hi = np.bincount(chunk[~lo_mask], minlength=NCH)
    n_tot = n_lo + n_hi
    start = np.zeros(NCH, np.int64)
    start[1:] = np.cumsum(n_tot)[:-1]

    t_lo_c = -(-n_lo // 128)  # ceil
    t_hi_c = -(-n_hi // 128)
    # slot profile: max tile count over the 8 cores for each of 49 slots
    TL = t_lo_c.reshape(N_CORES, C_PER_CORE).max(axis=0)
    TH = t_hi_c.reshape(N_CORES, C_PER_CORE).max(axis=0)
    TT = TL + TH
    T_total = int(TT.sum())
    S = T_total * 128  # total gather slots per core per layer

    x_pad = np.zeros((NP_, D_IN), np.float32)
    x_pad[:N] = x
    x_g16 = x_pad.astype(ml_dtypes.bfloat16)

    # inv_rep: [128, NODES_PC] per core, row-replicated inverse degree (bf16)
    # inv_colT: [128, C_PER_CORE] per core, per-dst-lane inverse degree (f32)
    # pair layout: for chunk pair (2q, 2q+1) the slot order is
    # [lo(2q) | lo(2q+1) | hi(2q) | hi(2q+1)] so one dma_gather covers both
    # chunks' lo (resp. hi) tiles.  Odd trailing chunk keeps [lo | hi].
    per_core = []
    for k in range(N_CORES):
        idx16 = np.zeros(S, np.int16)
        drel = np.full(S, 200.0, np.float32)
        pos = 0
        for q in range(0, C_PER_CORE, GROUP):
            js = list(range(q, min(q + GROUP, C_PER_CORE)))
            for j in js:  # lo blocks
                c = k * C_PER_CORE + j
                nl = int(n_lo[c])
                s0 = int(start[c])
                idx16[pos : pos + nl] = s_src[s0 : s0 + nl]
                drel[pos : pos + nl] = s_dst[s0 : s0 + nl] % CHUNK
                pos += int(TL[j]) * 128
            for j in js:  # hi blocks
                c = k * C_PER_CORE + j
                nl, nh = int(n_lo[c]), int(n_hi[c])
                s0 = int(start[c])
                idx16[pos : pos + nh] = s_src[s0 + nl : s0 + nl + nh] - SPLIT
                drel[pos : pos + nh] = s_dst[s0 + nl : s0 + nl + nh] % CHUNK
                pos += int(TH[j]) * 128
        assert pos == S
        idx_wrapped = np.ascontiguousarray(
            np.tile(idx16.reshape(S // 16, 16).T, (8, 1))
        )  # [128, S/16]
        drel2 = np.ascontiguousarray(
            drel.reshape(T_total, 128).T.astype(ml_dtypes.bfloat16)
        )  # [128, T]
        inv_k = inv[k * NODES_PC : (k + 1) * NODES_PC]
        inv_rep = np.ascontiguousarray(
            np.tile(inv_k[None, :], (128, 1)).astype(ml_dtypes.bfloat16)
        )  # [128, NODES_PC]
        inv_colT = np.ascontiguousarray(
            inv_k.reshape(C_PER_CORE, 128).T.astype(np.float32)
        )  # [128, 49]
        xT_k = np.ascontiguousarray(
            x_pad[k * NODES_PC : (k + 1) * NODES_PC].T.astype(ml_dtypes.bfloat16)
        )
        per_core.append(
            {
                "x_g": x_g16,
                "xT": xT_k,
                "idx": idx_wrapped,
                "drel": drel2,
                "inv_rep": inv_rep,
                "inv_colT": inv_colT,
            }
        )

    # ---- layer-2 tables: edges split by A/B block membership of the src ----
    src_r = s_src // NODES_PC
    src_off = s_src % NODES_PC
    isB = src_off >= ROWS_A
    rowAB = np.where(isB, src_r * ROWS_B + (src_off - ROWS_A),
                     src_r * ROWS_A + src_off)
    chunk_s = s_dst // CHUNK  # chunk of each edge in (chunk, lo/hi) order
    order2 = np.lexsort((isB, chunk_s))
    t_src = rowAB[order2]
    t_dst = s_dst[order2]
    t_isB = isB[order2]
    n_A = np.bincount(chunk_s[~isB], minlength=NCH)
    n_B = np.bincount(chunk_s[isB], minlength=NCH)
    TA = (-(-n_A // 128)).reshape(N_CORES, C_PER_CORE).max(axis=0)
    TB = (-(-n_B // 128)).reshape(N_CORES, C_PER_CORE).max(axis=0)
    T2 = int(TA.sum() + TB.sum())
    S2 = T2 * 128
    for k in range(N_CORES):
        idx16 = np.zeros(S2, np.int16)
        drel = np.full(S2, 200.0, np.float32)
        pos = 0
        for j in range(C_PER_CORE):
            c = k * C_PER_CORE + j
            na, nb = int(n_A[c]), int(n_B[c])
            s0 = int(start[c])
            idx16[pos : pos + na] = t_src[s0 : s0 + na]
            drel[pos : pos + na] = t_dst[s0 : s0 + na] % CHUNK
            assert not t_isB[s0 : s0 + na].any()
            pos += int(TA[j]) * 128
            idx16[pos : pos + nb] = t_src[s0 + na : s0 + na + nb]
            drel[pos : pos + nb] = t_dst[s0 + na : s0 + na + nb] % CHUNK
            assert t_isB[s0 + na : s0 + na + nb].all()
            pos += int(TB[j]) * 128
        assert pos == S2
        per_core[k]["idx2"] = np.ascontiguousarray(
            np.tile(idx16.reshape(S2 // 16, 16).T, (8, 1))
        )
        per_core[k]["drel2"] = np.ascontiguousarray(
            drel.reshape(T2, 128).T.astype(ml_dtypes.bfloat16)
        )
    return (
        per_core,
        [int(v) for v in TL],
        [int(v) for v in TH],
        [int(v) for v in TA],
        [int(v) for v in TB],
    )


def _shared_inputs(W1_l, b1, W1_r, W2_l, b2, W2_r):
    import ml_dtypes

    bf = ml_dtypes.bfloat16
    return {
        "W1_l": np.ascontiguousarray(np.asarray(W1_l, np.float32).astype(bf)),
        "W1_r": np.ascontiguousarray(np.asarray(W1_r, np.float32).astype(bf)),
        "W2_l": np.ascontiguousarray(np.asarray(W2_l, np.float32).astype(bf)),
        "W2_r": np.ascontiguousarray(np.asarray(W2_r, np.float32).astype(bf)),
        "b1": np.ascontiguousarray(np.asarray(b1, np.float32).reshape(D_HID, 1)),
        "b2": np.ascontiguousarray(
            np.asarray(b2, np.float32).astype(bf).reshape(1, D_OUT)
        ),
    }


def _build(TL, TH, TA, TB, n_chunks=C_PER_CORE):
    import concourse.bacc as bacc
    import concourse.mybir as mybir
    from concourse.tile import TileContext

    f32 = mybir.dt.float32
    bf16 = mybir.dt.bfloat16
    i16 = mybir.dt.int16
    TT = [a + b for a, b in zip(TL, TH)]
    T_total = sum(TT)
    S16 = T_total * 8  # idx table columns (16 idx per column)

    # chunk groups; col layout per group [lo..los | hi..his]
    pairs = [
        list(range(q, min(q + GROUP, n_chunks)))
        for q in range(0, n_chunks, GROUP)
    ]
    TT2 = [a + b for a, b in zip(TA, TB)]
    T2_total = sum(TT2)
    S16b = T2_total * 8
    PTmax = max(
        max(sum(TT[j] for j in js) for js in pairs),
        max(sum(TT2[j] for j in js) for js in pairs),
    )

    nc = bacc.Bacc(
        "TRN2",
        target_bir_lowering=False,
        debug=False,
        enable_asserts=False,
        num_devices=N_CORES,
    )

    x_g = nc.dram_tensor("x_g", [NP_, D_IN], bf16, kind="ExternalInput").ap()
    xT_d = nc.dram_tensor("xT", [128, NODES_PC], bf16, kind="ExternalInput").ap()
    idx_d = nc.dram_tensor("idx", [128, S16], i16, kind="ExternalInput").ap()
    drel_d = nc.dram_tensor("drel", [128, T_total], bf16, kind="ExternalInput").ap()
    invr_d = nc.dram_tensor(
        "inv_rep", [128, NODES_PC], bf16, kind="ExternalInput"
    ).ap()
    invc_d = nc.dram_tensor(
        "inv_colT", [128, C_PER_CORE], f32, kind="ExternalInput"
    ).ap()
    w1l_d = nc.dram_tensor("W1_l", [D_IN, D_HID], bf16, kind="ExternalInput").ap()
    w1r_d = nc.dram_tensor("W1_r", [D_IN, D_HID], bf16, kind="ExternalInput").ap()
    w2l_d = nc.dram_tensor("W2_l", [D_HID, D_OUT], bf16, kind="ExternalInput").ap()
    w2r_d = nc.dram_tensor("W2_r", [D_HID, D_OUT], bf16, kind="ExternalInput").ap()
    b1_d = nc.dram_tensor("b1", [D_HID, 1], f32, kind="ExternalInput").ap()
    b2_d = nc.dram_tensor("b2", [1, D_OUT], bf16, kind="ExternalInput").ap()
    idx2_d = nc.dram_tensor("idx2", [128, S16b], i16, kind="ExternalInput").ap()
    drel2_d = nc.dram_tensor(
        "drel2", [128, T2_total], bf16, kind="ExternalInput"
    ).ap()
    out_d = nc.dram_tensor("out", [NODES_PC, D_OUT], f32, kind="ExternalOutput").ap()
    p_full = nc.dram_tensor(
        "p_full", [NP_, 128], bf16, kind="Internal", addr_space="Shared"
    ).ap()

    relu = mybir.ActivationFunctionType.Relu
    is_eq = mybir.AluOpType.is_equal
    mult = mybir.AluOpType.mult
    add = mybir.AluOpType.add

    with TileContext(nc) as tc:
        with (
            tc.tile_pool(name="persist", bufs=1) as pp,
            tc.tile_pool(name="dram", bufs=1, space="DRAM") as dp,
            tc.tile_pool(name="msg", bufs=2) as mpool,
            tc.tile_pool(name="oh", bufs=3) as ohpool,
            tc.tile_pool(name="stage", bufs=3) as spool,
            tc.tile_pool(name="psA", bufs=2, space="PSUM") as psA,
            tc.tile_pool(name="psH", bufs=2, space="PSUM") as psH,
            tc.tile_pool(name="psO", bufs=2, space="PSUM") as psO,
        ):
            xT_sb = pp.tile([128, NODES_PC], bf16)
            nc.sync.dma_start(out=xT_sb[:], in_=xT_d)
            idx_sb = pp.tile([128, S16], i16)
            nc.sync.dma_start(out=idx_sb[:], in_=idx_d)
            drel_sb = pp.tile([128, T_total], bf16)
            nc.sync.dma_start(out=drel_sb[:], in_=drel_d)
            idx2_sb = pp.tile([128, S16b], i16)
            nc.sync.dma_start(out=idx2_sb[:], in_=idx2_d)
            drel2_sb = pp.tile([128, T2_total], bf16)
            nc.sync.dma_start(out=drel2_sb[:], in_=drel2_d)
            invr_sb = pp.tile([128, NODES_PC], bf16)
            nc.sync.dma_start(out=invr_sb[:], in_=invr_d)
            invc_sb = pp.tile([128, C_PER_CORE], f32)
            nc.sync.dma_start(out=invc_sb[:], in_=invc_d)
            w1l_sb = pp.tile([D_IN, D_HID], bf16)
            nc.sync.dma_start(out=w1l_sb[:], in_=w1l_d)
            w1r_sb = pp.tile([D_IN, D_HID], bf16)
            nc.sync.dma_start(out=w1r_sb[:], in_=w1r_d)
            w2l_sb = pp.tile([D_HID, D_OUT], bf16)
            nc.sync.dma_start(out=w2l_sb[:], in_=w2l_d)
            w2r_sb = pp.tile([D_HID, D_OUT], bf16)
            nc.sync.dma_start(out=w2r_sb[:], in_=w2r_d)
            b1_sb = pp.tile([D_HID, 1], f32)
            nc.sync.dma_start(out=b1_sb[:], in_=b1_d)
            b2_sb = pp.tile([1, D_OUT], bf16)
            nc.sync.dma_start(out=b2_sb[:], in_=b2_d)
            iota_sb = pp.tile([128, 128], f32)
            nc.gpsimd.iota(
                iota_sb[:],
                pattern=[[1, 128]],
                base=0,
                channel_multiplier=0,
                allow_small_or_imprecise_dtypes=True,
            )
            iota16 = pp.tile([128, 128], bf16)
            nc.vector.tensor_copy(out=iota16[:], in_=iota_sb[:])
            iota_rep = pp.tile([128, PTmax * 128], bf16)
            for t in range(PTmax):
                nc.scalar.copy(
                    out=iota_rep[:, t * 128 : (t + 1) * 128], in_=iota16[:]
                )
            ones_sb = pp.tile([1, 128], bf16)
            nc.vector.memset(ones_sb[:], 1.0)
            h_all = pp.tile([128, NODES_PC], bf16)
            p_bounce = dp.tile([NODES_PC, 128], bf16)

            # ---------------- phase A: layer 1 + p = h @ W2_l ----------------
            tb = 0
            for js in pairs:
                L = [TL[j] for j in js]
                H = [TH[j] for j in js]
                sl, sh = sum(L), sum(H)
                ttp = sl + sh
                msg = mpool.tile([128, PTmax * 128], bf16, tag="msg")
                off = 0
                for j in js:  # lo gathers per chunk (stay under ring limit)
                    if TL[j]:
                        nc.gpsimd.dma_gather(
                            out_ap=msg[:, off * 128 : (off + TL[j]) * 128].rearrange(
                                "p (t e) -> p t e", e=128
                            ),
                            in_ap=x_g[0:SPLIT, :],
                            idxs_ap=idx_sb[:, (tb + off) * 8 : (tb + off + TL[j]) * 8],
                            num_idxs=TL[j] * 128,
                            num_idxs_reg=TL[j] * 128,
                            elem_size=128,
                            single_packet=False,
                        )
                        off += TL[j]
                if sh:
                    nc.gpsimd.dma_gather(
                        out_ap=msg[:, sl * 128 : ttp * 128].rearrange(
                            "p (t e) -> p t e", e=128
                        ),
                        in_ap=x_g[SPLIT:NP_, :],
                        idxs_ap=idx_sb[:, (tb + sl) * 8 : (tb + ttp) * 8],
                        num_idxs=sh * 128,
                        num_idxs_reg=sh * 128,
                        elem_size=128,
                        single_packet=False,
                    )
                # batched one-hot build: oh[e, (t,d)] = (iota[d] == drel[e,t])
                oh = ohpool.tile([128, PTmax * 128], bf16, tag="oh")
                nc.vector.tensor_tensor(
                    out=oh[:, : ttp * 128].rearrange("p (t e) -> p t e", e=128),
                    in0=iota_rep[:, : ttp * 128].rearrange("p (t e) -> p t e", e=128),
                    in1=drel_sb[:, tb : tb + ttp]
                    .rearrange("p (t e) -> p t e", e=1)
                    .broadcast_to([128, ttp, 128]),
                    op=is_eq,
                )
                for i, j in enumerate(js):
                    # chunk j's tile columns within the pair buffer
                    cols = list(range(sum(L[:i]), sum(L[: i + 1]))) + list(
                        range(sl + sum(H[:i]), sl + sum(H[: i + 1]))
                    )
                    pa = psA.tile([128, 128], f32, tag="agg")
                    for ci, t in enumerate(cols):
                        nc.tensor.matmul(
                            out=pa[:],
                            lhsT=msg[:, t * 128 : (t + 1) * 128],
                            rhs=oh[:, t * 128 : (t + 1) * 128],
                            start=(ci == 0),
                            stop=(ci == len(cols) - 1),
                        )
                    # meanT[f, d] = aggT * inv_deg[d] (free-dim scale, inv_rep)
                    jsl = slice(j * 128, (j + 1) * 128)
                    meanT = spool.tile([128, 128], bf16, tag="meanT")
                    nc.vector.tensor_tensor(
                        out=meanT[:], in0=pa[:], in1=invr_sb[:, jsl], op=mult
                    )
                    ph = psH.tile([128, 128], f32, tag="h")
                    nc.tensor.matmul(
                        out=ph[:], lhsT=w1l_sb[:], rhs=meanT[:], start=True, stop=False
                    )
                    nc.tensor.matmul(
                        out=ph[:], lhsT=w1r_sb[:], rhs=xT_sb[:, jsl],
                        start=False, stop=True,
                    )
                    nc.scalar.activation(
                        out=h_all[:, jsl], in_=ph[:], func=relu,
                        bias=b1_sb[:, 0:1], scale=1.0,
                    )
                    po = psO.tile([128, D_OUT], f32, tag="p")
                    nc.tensor.matmul(
                        out=po[:], lhsT=h_all[:, jsl], rhs=w2l_sb[:],
                        start=True, stop=True,
                    )
                    p_sb = spool.tile([128, 128], bf16, tag="p_sb")
                    nc.vector.memset(p_sb[:, D_OUT:128], 0.0)
                    nc.scalar.copy(out=p_sb[:, 0:D_OUT], in_=po[:])
                    nc.sync.dma_start(out=p_bounce[jsl, :], in_=p_sb[:])
                tb += ttp

            # ---------------- all-gather p ----------------
            nc.gpsimd.collective_compute(
                "AllGather",
                mybir.AluOpType.bypass,
                replica_groups=[list(range(N_CORES))],
                ins=[p_bounce[:]],
                outs=[p_full],
            )

            # ---------------- phase B: layer 2 ----------------
            tb = 0
            for js in pairs:
                L = [TL[j] for j in js]
                H = [TH[j] for j in js]
                sl, sh = sum(L), sum(H)
                ttp = sl + sh
                msg2 = mpool.tile([128, PTmax * 128], bf16, tag="msg2")
                off = 0
                for j in js:  # lo gathers per chunk (stay under ring limit)
                    if TL[j]:
                        nc.gpsimd.dma_gather(
                            out_ap=msg2[:, off * 128 : (off + TL[j]) * 128].rearrange(
                                "p (t e) -> p t e", e=128
                            ),
                            in_ap=p_full[0:SPLIT, :],
                            idxs_ap=idx_sb[:, (tb + off) * 8 : (tb + off + TL[j]) * 8],
                            num_idxs=TL[j] * 128,
                            num_idxs_reg=TL[j] * 128,
                            elem_size=128,
                            single_packet=False,
                        )
                        off += TL[j]
                if sh:
                    nc.gpsimd.dma_gather(
                        out_ap=msg2[:, sl * 128 : ttp * 128].rearrange(
                            "p (t e) -> p t e", e=128
                        ),
                        in_ap=p_full[SPLIT:NP_, :],
                        idxs_ap=idx_sb[:, (tb + sl) * 8 : (tb + ttp) * 8],
                        num_idxs=sh * 128,
                        num_idxs_reg=sh * 128,
                        elem_size=128,
                        single_packet=False,
                    )
                oh = ohpool.tile([128, PTmax * 128], bf16, tag="oh")
                nc.vector.tensor_tensor(
                    out=oh[:, : ttp * 128].rearrange("p (t e) -> p t e", e=128),
                    in0=iota_rep[:, : ttp * 128].rearrange("p (t e) -> p t e", e=128),
                    in1=drel_sb[:, tb : tb + ttp]
                    .rearrange("p (t e) -> p t e", e=1)
                    .broadcast_to([128, ttp, 128]),
                    op=is_eq,
                )
                for i, j in enumerate(js):
                    cols = list(range(sum(L[:i]), sum(L[: i + 1]))) + list(
                        range(sl + sum(H[:i]), sl + sum(H[: i + 1]))
                    )
                    jsl = slice(j * 128, (j + 1) * 128)
                    # agg2[d, p-feat] = sum_e oh[e, d] * msg2[e, p]
                    pf = psA.tile([128, 128], f32, tag="fin")
                    for ci, t in enumerate(cols):
                        nc.tensor.matmul(
                            out=pf[:],
                            lhsT=oh[:, t * 128 : (t + 1) * 128],
                            rhs=msg2[:, t * 128 : (t + 1) * 128],
                            start=(ci == 0),
                            stop=(ci == len(cols) - 1),
                        )
                    # dense part: h @ W2_r + b2  -> pd [128d, 64]
                    pd = psO.tile([128, D_OUT], f32, tag="p")
                    nc.tensor.matmul(
                        out=pd[:], lhsT=h_all[:, jsl], rhs=w2r_sb[:],
                        start=True, stop=False,
                    )
                    nc.tensor.matmul(
                        out=pd[:], lhsT=ones_sb[:], rhs=b2_sb[:],
                        start=False, stop=True,
                    )
                    # out = pf[:, :64] * inv_col + pd
                    pd_sb = spool.tile([128, D_OUT], f32, tag="pd_sb")
                    nc.scalar.copy(out=pd_sb[:], in_=pd[:])
                    out_sb = spool.tile([128, D_OUT], f32, tag="out_sb")
                    nc.vector.scalar_tensor_tensor(
                        out=out_sb[:],
                        in0=pf[:, 0:D_OUT],
                        scalar=invc_sb[:, j : j + 1],
                        in1=pd_sb[:],
                        op0=mult,
                        op1=add,
                    )
                    nc.sync.dma_start(out=out_d[jsl, :], in_=out_sb[:])
                tb += ttp

    nc.compile()
    return nc


def kernel(
    x,
    edge_index,
    W1_l,
    b1,
    W1_r,
    W2_l,
    b2,
    W2_r,
):
    from concourse.bass_utils import run_bass_kernel_spmd

    per_core, TL, TH, TA, TB = _preprocess(x, edge_index)
    nc = _build(TL, TH, TA, TB)

    shared = _shared_inputs(W1_l, b1, W1_r, W2_l, b2, W2_r)
    in_maps = [{**pc, **shared} for pc in per_core]

    res = run_bass_kernel_spmd(nc, in_maps, core_ids=list(range(N_CORES)))
    out = np.concatenate([r["out"] for r in res.results], axis=0)
    return out[:N].astype(np.float32)


if __name__ == "__main__":
    rng = np.random.default_rng(0)
    x = rng.standard_normal((N, D_IN), dtype=np.float32)
    ei = rng.integers(0, N, size=(2, E), dtype=np.int64)
    s = 1.0 / np.sqrt(D_IN)
    w1l = rng.uniform(-s, s, (D_IN, D_HID)).astype(np.float32)
    w1r = rng.uniform(-s, s, (D_IN, D_HID)).astype(np.float32)
    s2 = 1.0 / np.sqrt(D_HID)
    w2l = rng.uniform(-s2, s2, (D_HID, D_OUT)).astype(np.float32)
    w2r = rng.uniform(-s2, s2, (D_HID, D_OUT)).astype(np.float32)
    out = kernel(
        x=x,
        edge_index=ei,
        W1_l=w1l,
        b1=np.zeros(D_HID, np.float32),
        W1_r=w1r,
        W2_l=w2l,
        b2=np.zeros(D_OUT, np.float32),
        W2_r=w2r,
    )
    print(out.shape, out.dtype)


# revision 28
# speedup vs baseline: 1.1908x; 1.0191x over previous
"""GraphSAGE 2-layer forward on 8 Trainium2 NeuronCores.

Strategy (sharding_hint: partition edges by destination node):
  - Nodes are padded to NP=50176 = 8 cores * 49 chunks * 128 and sharded by
    destination across the 8 cores (6272 dst nodes per core).
  - Edges are bucketed by dst chunk (128 dst nodes per chunk).  Within a
    bucket, edges are split into src<32768 ("lo") and src>=32768 ("hi")
    sub-lists because dma_gather indices are int16.
  - Per chunk, messages x[src] are fetched with dma_gather (one instruction
    per lo/hi block) in bf16 (256B descriptors), and aggregated with one-hot
    matmuls on the tensor engine.  One-hot (0/1) selection matrices for a
    whole chunk (up to TTmax tiles) are built in a single batched DVE
    tensor_tensor is_equal op against a broadcast drel table; the 1/deg mean
    scaling is applied after aggregation (free-dim inv_rep multiply for
    layer 1, per-partition scalar multiply for layer 2).
  - Layer 2 gathers h @ W2_l (= p, 64 wide) stored padded to 128 bf16 lanes
    so the same int16 index tables are reused; p is exchanged between cores
    with an AllGather.
  - All compute matmuls run in bf16 (messages, one-hots, weights) with fp32
    PSUM accumulation.
"""

import sys

sys.path.insert(0, "/opt/trn_rl_repo")

import numpy as np

N = 50000
E = 800000
D_IN, D_HID, D_OUT = 128, 128, 64
N_CORES = 8
CHUNK = 128
C_PER_CORE = 49
NODES_PC = C_PER_CORE * CHUNK  # 6272
NP_ = N_CORES * NODES_PC  # 50176
NCH = N_CORES * C_PER_CORE  # 392 chunks
SPLIT = 32768
GROUP = 1  # chunks per slot-layout group. Measured: per-chunk gathers
# (GROUP=1, ~1340/~710 idx) beat merged variants; Q7 desc-gen is purely
# per-descriptor at these sizes so merging instructions saves nothing and
# larger gathers run at a worse marginal rate.
# layer-2 p exchange is split into two AllGathers so the first overlaps
# phase A: block A = local chunks [0, CA), block B = [CA, 49)
CA = 25
ROWS_A = CA * CHUNK  # 3200 rows per core
ROWS_B = NODES_PC - ROWS_A  # 3072


def _preprocess(x, edge_index):
    """Host-side index preprocessing. Returns per-core input maps + profile."""
    import ml_dtypes

    x = np.asarray(x, dtype=np.float32)
    src = np.asarray(edge_index[0], dtype=np.int64)
    dst = np.asarray(edge_index[1], dtype=np.int64)

    cnt = np.bincount(dst, minlength=NP_).astype(np.float32)
    inv = (1.0 / np.maximum(cnt, 1.0)).astype(np.float32)

    chunk = dst // CHUNK
    hi_flag = (src >= SPLIT).astype(np.int64)
    order = np.lexsort((hi_flag, chunk))
    s_src = src[order]
    s_dst = dst[order]

    lo_mask = src < SPLIT
    n_lo = np.bincount(chunk[lo_mask], minlength=NCH)
    n_hi = np.bincount(chunk[~lo_mask], minlength=NCH)
    n_tot = n_lo + n_hi
    start = np.zeros(NCH, np.int64)
    start[1:] = np.cumsum(n_tot)[:-1]

    t_lo_c = -(-n_lo // 128)  # ceil
    t_hi_c = -(-n_hi // 128)
    # slot profile: max tile count over the 8 cores for each of 49 slots
    TL = t_lo_c.reshape(N_CORES, C_PER_CORE).max(axis=0)
    TH = t_hi_c.reshape(N_CORES, C_PER_CORE).max(axis=0)
    TT = TL + TH
    T_total = int(TT.sum())
    S = T_total * 128  # total gather slots per core per layer

    x_pad = np.zeros((NP_, D_IN), np.float32)
    x_pad[:N] = x
    x_g16 = x_pad.astype(ml_dtypes.bfloat16)

    # inv_rep: [128, NODES_PC] per core, row-replicated inverse degree (bf16)
    # inv_colT: [128, C_PER_CORE] per core, per-dst-lane inverse degree (f32)
    # pair layout: for chunk pair (2q, 2q+1) the slot order is
    # [lo(2q) | lo(2q+1) | hi(2q) | hi(2q+1)] so one dma_gather covers both
    # chunks' lo (resp. hi) tiles.  Odd trailing chunk keeps [lo | hi].
    per_core = []
    for k in range(N_CORES):
        idx16 = np.zeros(S, np.int16)
        drel = np.full(S, 200.0, np.float32)
        pos = 0
        for q in range(0, C_PER_CORE, GROUP):
            js = list(range(q, min(q + GROUP, C_PER_CORE)))
            for j in js:  # lo blocks
                c = k * C_PER_CORE + j
                nl = int(n_lo[c])
                s0 = int(start[c])
                idx16[pos : pos + nl] = s_src[s0 : s0 + nl]
                drel[pos : pos + nl] = s_dst[s0 : s0 + nl] % CHUNK
                pos += int(TL[j]) * 128
            for j in js:  # hi blocks
                c = k * C_PER_CORE + j
                nl, nh = int(n_lo[c]), int(n_hi[c])
                s0 = int(start[c])
                idx16[pos : pos + nh] = s_src[s0 + nl : s0 + nl + nh] - SPLIT
                drel[pos : pos + nh] = s_dst[s0 + nl : s0 + nl + nh] % CHUNK
                pos += int(TH[j]) * 128
        assert pos == S
        idx_wrapped = np.ascontiguousarray(
            np.tile(idx16.reshape(S // 16, 16).T, (8, 1))
        )  # [128, S/16]
        drel2 = np.ascontiguousarray(
            drel.reshape(T_total, 128).T.astype(ml_dtypes.bfloat16)
        )  # [128, T]
        inv_k = inv[k * NODES_PC : (k + 1) * NODES_PC]
        inv_rep = np.ascontiguousarray(
            np.tile(inv_k[None, :], (128, 1)).astype(ml_dtypes.bfloat16)
        )  # [128, NODES_PC]
        inv_colT = np.ascontiguousarray(
            inv_k.reshape(C_PER_CORE, 128).T.astype(np.float32)
        )  # [128, 49]
        xT_k = np.ascontiguousarray(
            x_pad[k * NODES_PC : (k + 1) * NODES_PC].T.astype(ml_dtypes.bfloat16)
        )
        per_core.append(
            {
                "x_g": x_g16,
                "xT": xT_k,
                "idx": idx_wrapped,
                "drel": drel2,
                "inv_rep": inv_rep,
                "inv_colT": inv_colT,
            }
        )

    # ---- layer-2 tables: edges split by A/B block membership of the src ----
    src_r = s_src // NODES_PC
    src_off = s_src % NODES_PC
    isB = src_off >= ROWS_A
    rowAB = np.where(isB, src_r * ROWS_B + (src_off - ROWS_A),
                     src_r * ROWS_A + src_off)
    chunk_s = s_dst // CHUNK  # chunk of each edge in (chunk, lo/hi) order
    order2 = np.lexsort((isB, chunk_s))
    t_src = rowAB[order2]
    t_dst = s_dst[order2]
    t_isB = isB[order2]
    n_A = np.bincount(chunk_s[~isB], minlength=NCH)
    n_B = np.bincount(chunk_s[isB], minlength=NCH)
    TA = (-(-n_A // 128)).reshape(N_CORES, C_PER_CORE).max(axis=0)
    TB = (-(-n_B // 128)).reshape(N_CORES, C_PER_CORE).max(axis=0)
    T2 = int(TA.sum() + TB.sum())
    S2 = T2 * 128
    for k in range(N_CORES):
        idx16 = np.zeros(S2, np.int16)
        drel = np.full(S2, 200.0, np.float32)
        pos = 0
        for j in range(C_PER_CORE):
            c = k * C_PER_CORE + j
            na, nb = int(n_A[c]), int(n_B[c])
            s0 = int(start[c])
            idx16[pos : pos + na] = t_src[s0 : s0 + na]
            drel[pos : pos + na] = t_dst[s0 : s0 + na] % CHUNK
            assert not t_isB[s0 : s0 + na].any()
            pos += int(TA[j]) * 128
            idx16[pos : pos + nb] = t_src[s0 + na : s0 + na + nb]
            drel[pos : pos + nb] = t_dst[s0 + na : s0 + na + nb] % CHUNK
            assert t_isB[s0 + na : s0 + na + nb].all()
            pos += int(TB[j]) * 128
        assert pos == S2
        per_core[k]["idx2"] = np.ascontiguousarray(
            np.tile(idx16.reshape(S2 // 16, 16).T, (8, 1))
        )
        per_core[k]["drel2"] = np.ascontiguousarray(
            drel.reshape(T2, 128).T.astype(ml_dtypes.bfloat16)
        )
    return (
        per_core,
        [int(v) for v in TL],
        [int(v) for v in TH],
        [int(v) for v in TA],
        [int(v) for v in TB],
    )


def _shared_inputs(W1_l, b1, W1_r, W2_l, b2, W2_r):
    import ml_dtypes

    bf = ml_dtypes.bfloat16
    return {
        "W1_l": np.ascontiguousarray(np.asarray(W1_l, np.float32).astype(bf)),
        "W1_r": np.ascontiguousarray(np.asarray(W1_r, np.float32).astype(bf)),
        "W2_l": np.ascontiguousarray(np.asarray(W2_l, np.float32).astype(bf)),
        "W2_r": np.ascontiguousarray(np.asarray(W2_r, np.float32).astype(bf)),
        "b1": np.ascontiguousarray(np.asarray(b1, np.float32).reshape(D_HID, 1)),
        "b2": np.ascontiguousarray(
            np.asarray(b2, np.float32).astype(bf).reshape(1, D_OUT)
        ),
    }


def _build(TL, TH, TA, TB, n_chunks=C_PER_CORE):
    import concourse.bacc as bacc
    import concourse.mybir as mybir
    from concourse.tile import TileContext

    f32 = mybir.dt.float32
    bf16 = mybir.dt.bfloat16
    i16 = mybir.dt.int16
    TT = [a + b for a, b in zip(TL, TH)]
    T_total = sum(TT)
    S16 = T_total * 8  # idx table columns (16 idx per column)

    # chunk groups; col layout per group [lo..los | hi..his]
    pairs = [
        list(range(q, min(q + GROUP, n_chunks)))
        for q in range(0, n_chunks, GROUP)
    ]
    TT2 = [a + b for a, b in zip(TA, TB)]
    T2_total = sum(TT2)
    S16b = T2_total * 8
    PTmax = max(
        max(sum(TT[j] for j in js) for js in pairs),
        max(sum(TT2[j] for j in js) for js in pairs),
    )

    nc = bacc.Bacc(
        "TRN2",
        target_bir_lowering=False,
        debug=False,
        enable_asserts=False,
        num_devices=N_CORES,
    )

    x_g = nc.dram_tensor("x_g", [NP_, D_IN], bf16, kind="ExternalInput").ap()
    xT_d = nc.dram_tensor("xT", [128, NODES_PC], bf16, kind="ExternalInput").ap()
    idx_d = nc.dram_tensor("idx", [128, S16], i16, kind="ExternalInput").ap()
    drel_d = nc.dram_tensor("drel", [128, T_total], bf16, kind="ExternalInput").ap()
    invr_d = nc.dram_tensor(
        "inv_rep", [128, NODES_PC], bf16, kind="ExternalInput"
    ).ap()
    invc_d = nc.dram_tensor(
        "inv_colT", [128, C_PER_CORE], f32, kind="ExternalInput"
    ).ap()
    w1l_d = nc.dram_tensor("W1_l", [D_IN, D_HID], bf16, kind="ExternalInput").ap()
    w1r_d = nc.dram_tensor("W1_r", [D_IN, D_HID], bf16, kind="ExternalInput").ap()
    w2l_d = nc.dram_tensor("W2_l", [D_HID, D_OUT], bf16, kind="ExternalInput").ap()
    w2r_d = nc.dram_tensor("W2_r", [D_HID, D_OUT], bf16, kind="ExternalInput").ap()
    b1_d = nc.dram_tensor("b1", [D_HID, 1], f32, kind="ExternalInput").ap()
    b2_d = nc.dram_tensor("b2", [1, D_OUT], bf16, kind="ExternalInput").ap()
    idx2_d = nc.dram_tensor("idx2", [128, S16b], i16, kind="ExternalInput").ap()
    drel2_d = nc.dram_tensor(
        "drel2", [128, T2_total], bf16, kind="ExternalInput"
    ).ap()
    out_d = nc.dram_tensor("out", [NODES_PC, D_OUT], f32, kind="ExternalOutput").ap()
    p_full = nc.dram_tensor(
        "p_full", [NP_, 128], bf16, kind="Internal", addr_space="Shared"
    ).ap()

    relu = mybir.ActivationFunctionType.Relu
    is_eq = mybir.AluOpType.is_equal
    mult = mybir.AluOpType.mult
    add = mybir.AluOpType.add

    with TileContext(nc) as tc:
        with (
            tc.tile_pool(name="persist", bufs=1) as pp,
            tc.tile_pool(name="dram", bufs=1, space="DRAM") as dp,
            tc.tile_pool(name="msg", bufs=2) as mpool,
            tc.tile_pool(name="oh", bufs=3) as ohpool,
            tc.tile_pool(name="stage", bufs=3) as spool,
            tc.tile_pool(name="psA", bufs=2, space="PSUM") as psA,
            tc.tile_pool(name="psH", bufs=2, space="PSUM") as psH,
            tc.tile_pool(name="psO", bufs=2, space="PSUM") as psO,
        ):
            xT_sb = pp.tile([128, NODES_PC], bf16)
            nc.sync.dma_start(out=xT_sb[:], in_=xT_d)
            idx_sb = pp.tile([128, S16], i16)
            nc.sync.dma_start(out=idx_sb[:], in_=idx_d)
            drel_sb = pp.tile([128, T_total], bf16)
            nc.sync.dma_start(out=drel_sb[:], in_=drel_d)
            idx2_sb = pp.tile([128, S16b], i16)
            nc.sync.dma_start(out=idx2_sb[:], in_=idx2_d)
            drel2_sb = pp.tile([128, T2_total], bf16)
            nc.sync.dma_start(out=drel2_sb[:], in_=drel2_d)
            invr_sb = pp.tile([128, NODES_PC], bf16)
            nc.sync.dma_start(out=invr_sb[:], in_=invr_d)
            invc_sb = pp.tile([128, C_PER_CORE], f32)
            nc.sync.dma_start(out=invc_sb[:], in_=invc_d)
            w1l_sb = pp.tile([D_IN, D_HID], bf16)
            nc.sync.dma_start(out=w1l_sb[:], in_=w1l_d)
            w1r_sb = pp.tile([D_IN, D_HID], bf16)
            nc.sync.dma_start(out=w1r_sb[:], in_=w1r_d)
            w2l_sb = pp.tile([D_HID, D_OUT], bf16)
            nc.sync.dma_start(out=w2l_sb[:], in_=w2l_d)
            w2r_sb = pp.tile([D_HID, D_OUT], bf16)
            nc.sync.dma_start(out=w2r_sb[:], in_=w2r_d)
            b1_sb = pp.tile([D_HID, 1], f32)
            nc.sync.dma_start(out=b1_sb[:], in_=b1_d)
            b2_sb = pp.tile([1, D_OUT], bf16)
            nc.sync.dma_start(out=b2_sb[:], in_=b2_d)
            iota_sb = pp.tile([128, 128], f32)
            nc.gpsimd.iota(
                iota_sb[:],
                pattern=[[1, 128]],
                base=0,
                channel_multiplier=0,
                allow_small_or_imprecise_dtypes=True,
            )
            iota16 = pp.tile([128, 128], bf16)
            nc.vector.tensor_copy(out=iota16[:], in_=iota_sb[:])
            iota_rep = pp.tile([128, PTmax * 128], bf16)
            for t in range(PTmax):
                nc.scalar.copy(
                    out=iota_rep[:, t * 128 : (t + 1) * 128], in_=iota16[:]
                )
            ones_sb = pp.tile([1, 128], bf16)
            nc.vector.memset(ones_sb[:], 1.0)
            h_all = pp.tile([128, NODES_PC], bf16)
            p_bounce = dp.tile([NODES_PC, 128], bf16)

            # ---------------- phase A: layer 1 + p = h @ W2_l ----------------
            tb = 0
            for js in pairs:
                L = [TL[j] for j in js]
                H = [TH[j] for j in js]
                sl, sh = sum(L), sum(H)
                ttp = sl + sh
                msg = mpool.tile([128, PTmax * 128], bf16, tag="msg")
                off = 0
                for j in js:  # lo gathers per chunk (stay under ring limit)
                    if TL[j]:
                        nc.gpsimd.dma_gather(
                            out_ap=msg[:, off * 128 : (off + TL[j]) * 128].rearrange(
                                "p (t e) -> p t e", e=128
                            ),
                            in_ap=x_g[0:SPLIT, :],
                            idxs_ap=idx_sb[:, (tb + off) * 8 : (tb + off + TL[j]) * 8],
                            num_idxs=TL[j] * 128,
                            num_idxs_reg=TL[j] * 128,
                            elem_size=128,
                            single_packet=False,
                        )
                        off += TL[j]
                if sh:
                    nc.gpsimd.dma_gather(
                        out_ap=msg[:, sl * 128 : ttp * 128].rearrange(
                            "p (t e) -> p t e", e=128
                        ),
                        in_ap=x_g[SPLIT:NP_, :],
                        idxs_ap=idx_sb[:, (tb + sl) * 8 : (tb + ttp) * 8],
                        num_idxs=sh * 128,
                        num_idxs_reg=sh * 128,
                        elem_size=128,
                        single_packet=False,
                    )
                # batched one-hot build: oh[e, (t,d)] = (iota[d] == drel[e,t])
                oh = ohpool.tile([128, PTmax * 128], bf16, tag="oh")
                nc.vector.tensor_tensor(
                    out=oh[:, : ttp * 128].rearrange("p (t e) -> p t e", e=128),
                    in0=iota_rep[:, : ttp * 128].rearrange("p (t e) -> p t e", e=128),
                    in1=drel_sb[:, tb : tb + ttp]
                    .rearrange("p (t e) -> p t e", e=1)
                    .broadcast_to([128, ttp, 128]),
                    op=is_eq,
                )
                for i, j in enumerate(js):
                    # chunk j's tile columns within the pair buffer
                    cols = list(range(sum(L[:i]), sum(L[: i + 1]))) + list(
                        range(sl + sum(H[:i]), sl + sum(H[: i + 1]))
                    )
                    pa = psA.tile([128, 128], f32, tag="agg")
                    for ci, t in enumerate(cols):
                        nc.tensor.matmul(
                            out=pa[:],
                            lhsT=msg[:, t * 128 : (t + 1) * 128],
                            rhs=oh[:, t * 128 : (t + 1) * 128],
                            start=(ci == 0),
                            stop=(ci == len(cols) - 1),
                        )
                    # meanT[f, d] = aggT * inv_deg[d] (free-dim scale, inv_rep)
                    jsl = slice(j * 128, (j + 1) * 128)
                    meanT = spool.tile([128, 128], bf16, tag="meanT")
                    nc.vector.tensor_tensor(
                        out=meanT[:], in0=pa[:], in1=invr_sb[:, jsl], op=mult
                    )
                    ph = psH.tile([128, 128], f32, tag="h")
                    nc.tensor.matmul(
                        out=ph[:], lhsT=w1l_sb[:], rhs=meanT[:], start=True, stop=False
                    )
                    nc.tensor.matmul(
                        out=ph[:], lhsT=w1r_sb[:], rhs=xT_sb[:, jsl],
                        start=False, stop=True,
                    )
                    nc.scalar.activation(
                        out=h_all[:, jsl], in_=ph[:], func=relu,
                        bias=b1_sb[:, 0:1], scale=1.0,
                    )
                    po = psO.tile([128, D_OUT], f32, tag="p")
                    nc.tensor.matmul(
                        out=po[:], lhsT=h_all[:, jsl], rhs=w2l_sb[:],
                        start=True, stop=True,
                    )
                    p_sb = spool.tile([128, 128], bf16, tag="p_sb")
                    nc.vector.memset(p_sb[:, D_OUT:128], 0.0)
                    nc.scalar.copy(out=p_sb[:, 0:D_OUT], in_=po[:])
                    nc.sync.dma_start(out=p_bounce[jsl, :], in_=p_sb[:])
                tb += ttp

            # ---------------- all-gather p ----------------
            nc.gpsimd.collective_compute(
                "AllGather",
                mybir.AluOpType.bypass,
                replica_groups=[list(range(N_CORES))],
                ins=[p_bounce[:]],
                outs=[p_full],
            )

            # ---------------- phase B: layer 2 ----------------
            tb = 0
            for js in pairs:
                L = [TL[j] for j in js]
                H = [TH[j] for j in js]
                sl, sh = sum(L), sum(H)
                ttp = sl + sh
                msg2 = mpool.tile([128, PTmax * 128], bf16, tag="msg2")
                off = 0
                for j in js:  # lo gathers per chunk (stay under ring limit)
                    if TL[j]:
                        nc.gpsimd.dma_gather(
                            out_ap=msg2[:, off * 128 : (off + TL[j]) * 128].rearrange(
                                "p (t e) -> p t e", e=128
                            ),
                            in_ap=p_full[0:SPLIT, :],
                            idxs_ap=idx_sb[:, (tb + off) * 8 : (tb + off + TL[j]) * 8],
                            num_idxs=TL[j] * 128,
                            num_idxs_reg=TL[j] * 128,
                            elem_size=128,
                            single_packet=False,
                        )
                        off += TL[j]
                if sh:
                    nc.gpsimd.dma_gather(
                        out_ap=msg2[:, sl * 128 : ttp * 128].rearrange(
                            "p (t e) -> p t e", e=128
                        ),
                        in_ap=p_full[SPLIT:NP_, :],
                        idxs_ap=idx_sb[:, (tb + sl) * 8 : (tb + ttp) * 8],
                        num_idxs=sh * 128,
                        num_idxs_reg=sh * 128,
                        elem_size=128,
                        single_packet=False,
                    )
                oh = ohpool.tile([128, PTmax * 128], bf16, tag="oh")
                nc.vector.tensor_tensor(
                    out=oh[:, : ttp * 128].rearrange("p (t e) -> p t e", e=128),
                    in0=iota_rep[:, : ttp * 128].rearrange("p (t e) -> p t e", e=128),
                    in1=drel_sb[:, tb : tb + ttp]
                    .rearrange("p (t e) -> p t e", e=1)
                    .broadcast_to([128, ttp, 128]),
                    op=is_eq,
                )
                for i, j in enumerate(js):
                    cols = list(range(sum(L[:i]), sum(L[: i + 1]))) + list(
                        range(sl + sum(H[:i]), sl + sum(H[: i + 1]))
                    )
                    jsl = slice(j * 128, (j + 1) * 128)
                    # agg2[d, p-feat] = sum_e oh[e, d] * msg2[e, p]
                    pf = psA.tile([128, 128], f32, tag="fin")
                    for ci, t in enumerate(cols):
                        nc.tensor.matmul(
                            out=pf[:],
                            lhsT=oh[:, t * 128 : (t + 1) * 128],
                            rhs=msg2[:, t * 128 : (t + 1) * 128],
                            start=(ci == 0),
                            stop=(ci == len(cols) - 1),
                        )
                    # dense part: h @ W2_r + b2  -> pd [128d, 64]
                    pd = psO.tile([128, D_OUT], f32, tag="p")
                    nc.tensor.matmul(
                        out=pd[:], lhsT=h_all[:, jsl], rhs=w2r_sb[:],
                        start=True, stop=False,
                    )
                    nc.tensor.matmul(
                        out=pd[:], lhsT=ones_sb[:], rhs=b2_sb[:],
                        start=False, stop=True,
                    )
                    # out = pf[:, :64] * inv_col + pd
                    pd_sb = spool.tile([128, D_OUT], f32, tag="pd_sb")
                    nc.scalar.copy(out=pd_sb[:], in_=pd[:])
                    out_sb = spool.tile([128, D_OUT], f32, tag="out_sb")
                    nc.vector.scalar_tensor_tensor(
                        out=out_sb[:],
                        in0=pf[:, 0:D_OUT],
                        scalar=invc_sb[:, j : j + 1],
                        in1=pd_sb[:],
                        op0=mult,
                        op1=add,
                    )
                    nc.sync.dma_start(out=out_d[jsl, :], in_=out_sb[:])
                tb += ttp

    nc.compile()
    return nc


def kernel(
    x,
    edge_index,
    W1_l,
    b1,
    W1_r,
    W2_l,
    b2,
    W2_r,
):
    from concourse.bass_utils import run_bass_kernel_spmd

    per_core, TL, TH, TA, TB = _preprocess(x, edge_index)
    nc = _build(TL, TH, TA, TB)

    shared = _shared_inputs(W1_l, b1, W1_r, W2_l, b2, W2_r)
    in_maps = [{**pc, **shared} for pc in per_core]

    res = run_bass_kernel_spmd(nc, in_maps, core_ids=list(range(N_CORES)))
    out = np.concatenate([r["out"] for r in res.results], axis=0)
    return out[:N].astype(np.float32)


if __name__ == "__main__":
    rng = np.random.default_rng(0)
    x = rng.standard_normal((N, D_IN), dtype=np.float32)
    ei = rng.integers(0, N, size=(2, E), dtype=np.int64)
    s = 1.0 / np.sqrt(D_IN)
    w1l = rng.uniform(-s, s, (D_IN, D_HID)).astype(np.float32)
    w1r = rng.uniform(-s, s, (D_IN, D_HID)).astype(np.float32)
    s2 = 1.0 / np.sqrt(D_HID)
    w2l = rng.uniform(-s2, s2, (D_HID, D_OUT)).astype(np.float32)
    w2r = rng.uniform(-s2, s2, (D_HID, D_OUT)).astype(np.float32)
    out = kernel(
        x=x,
        edge_index=ei,
        W1_l=w1l,
        b1=np.zeros(D_HID, np.float32),
        W1_r=w1r,
        W2_l=w2l,
        b2=np.zeros(D_OUT, np.float32),
        W2_r=w2r,
    )
    print(out.shape, out.dtype)
